# revision 1
# baseline (speedup 1.0000x reference)
"""DeepFM forward kernel for 8 Trainium2 NeuronCores (Bass/Tile).

Math (per batch row b):
    lin[b] = x[b] @ w + b0
    C[b]   = sum_k (x[b] @ v)_k^2
    Bq[b]  = sum_f s[f] * x[b,f]^2,   s[f] = sum_k v[f,k]^2
    out[b] = sigmoid(lin[b] + 0.5*C[b] - 0.5*Bq[b])

Data-parallel: batch 16384 sharded 8 ways (2048 rows/core); parameters
replicated. x is shipped pre-transposed (features on partitions) so every
matmul contracts over the partition dim with no on-chip transposes.

Precision scheme (hardware fp32r truncates matmul inputs to 11 mantissa
bits; engine writes to f32r tiles round to the same grid):
  - A-stream (xv + lin): 3 fp32r passes  x11@vw11 + x11@vwl + xl@vw11
    where x11 = round11(x), xl = x - x11 (exact), vw split likewise.
    Residual ~2^-22 relative — fp32-level.
  - B-stream (PRECISE_B): 2 fp32r passes over m = s*x^2 (ACT Square with
    per-feature sqrt(s) scale): hi = round11(m) and the exact residual
    m - hi, accumulated into the same PSUM row. End-to-end output error is
    at the fp32 reference's own noise floor (~1e-6 norm rel).
    With PRECISE_B=False: single truncated pass, ~2e-4 absmax, ~15% faster.
"""

import numpy as np

import concourse.bass as bass
import concourse.tile as tile
from concourse import bacc, mybir
from concourse.bass_utils import run_bass_kernel_spmd

BATCH, FIELD, EMBED = 16384, 2048, 64
NCORES = 8
BS = BATCH // NCORES   # 2048 batch rows per core
NCHUNK = 512           # psum free-dim per matmul
KTILES = FIELD // 128  # 16 contraction tiles
NCHUNKS = BS // NCHUNK  # 4 batch chunks per core
M = EMBED + 1          # 65 stationary columns: v plus w

F32 = mybir.dt.float32
F32R = mybir.dt.float32r
AF = mybir.ActivationFunctionType

# Two-pass B-stream: adds an exact-residual pass for the quadratic term,
# taking the output to fp32-reference accuracy (~1e-7) at ~10% more time.
PRECISE_B = True


def _build_nc():
    nc = bacc.Bacc("TRN2", target_bir_lowering=False, debug=False)

    xt = nc.declare_dram_parameter("xt", [FIELD, BS], F32, isOutput=False)
    # host-packed SBUF images: [128, KTILES*M], [128, KTILES]
    vw11i = nc.declare_dram_parameter("vw11i", [128, KTILES * M], F32R, isOutput=False)
    vwli = nc.declare_dram_parameter("vwli", [128, KTILES * M], F32R, isOutput=False)
    sqsi = nc.declare_dram_parameter("sqsi", [128, KTILES], F32, isOutput=False)
    red = nc.declare_dram_parameter("red", [97, 1], F32, isOutput=False)
    ones = nc.declare_dram_parameter("ones", [128, 1], F32R, isOutput=False)
    bvec = nc.declare_dram_parameter("bvec", [1, 1], F32, isOutput=False)
    y = nc.declare_dram_parameter("y", [NCHUNKS, NCHUNK], F32, isOutput=True)

    with tile.TileContext(nc) as tc:
        with (
            tc.tile_pool(name="consts", bufs=1) as consts,
            tc.tile_pool(name="xin", bufs=5) as xin,
            tc.tile_pool(name="x11p", bufs=5) as x11p,
            tc.tile_pool(name="xlp", bufs=4) as xlp,
            tc.tile_pool(name="mfp", bufs=3) as mfp,
            tc.tile_pool(name="mrp", bufs=3) as mrp,
            tc.tile_pool(name="mlp", bufs=3) as mlp,
            tc.tile_pool(name="redrhs", bufs=4) as redrhs,
            tc.tile_pool(name="outp", bufs=2) as outp,
            tc.tile_pool(name="psA", bufs=NCHUNKS, space="PSUM") as psA,
            tc.tile_pool(name="psB", bufs=NCHUNKS, space="PSUM") as psB,
        ):
            # ---- replicated parameters, loaded once. All consts ride the
            # ACT queue so SP streams x and Pool starts x11 copies at t=0;
            # the ones DMA is issued after the first stripe (see below) so it
            # doesn't block Pool's first x11 copy. ----
            vw11 = consts.tile([128, KTILES * M], F32R)
            nc.gpsimd.dma_start(vw11[:, :], vw11i[:, :])
            sqs_sb = consts.tile([128, KTILES], F32)
            nc.scalar.dma_start(sqs_sb[:, :], sqsi[:, :])
            ones_sb = consts.tile([128, 1], F32R)
            nc.gpsimd.dma_start(ones_sb[:, :], ones[:, :])
            vwl = consts.tile([128, KTILES * M], F32R)
            nc.scalar.dma_start(vwl[:, :], vwli[:, :])
            red_sb = consts.tile([97, 1], F32)
            nc.scalar.dma_start(red_sb[:, :], red[:, :])
            b_sb = consts.tile([1, 1], F32)
            nc.scalar.dma_start(b_sb[:, :], bvec[:, :])

            psumA = [
                psA.tile([M, NCHUNK], F32, name=f"psumA{n}", tag="psumA")
                for n in range(NCHUNKS)
            ]
            psumB = [
                psB.tile([1, NCHUNK], F32, name=f"psumB{n}", tag="psumB")
                for n in range(NCHUNKS)
            ]

            def process(k, pieces):
                """One contraction stripe k, split into `pieces` column blocks
                (list of (col_lo, col_hi)); each block covers whole chunks."""
                vw11_k = vw11[:, k * M:(k + 1) * M]
                vwl_k = vwl[:, k * M:(k + 1) * M]
                first, last = k == 0, k == KTILES - 1
                for lo, hi in pieces:
                    w = hi - lo
                    xk = xin.tile([128, w], F32, name=f"xk{k}_{lo}", tag="xk")
                    nc.sync.dma_start(xk[:, :], xt[k * 128:(k + 1) * 128, lo:hi])
                    # Engine balance: DVE is the busiest engine (the two
                    # full-rate f32 subs); hand a 128-col slice of each sub
                    # to GPSIMD, which has slack.
                    spl = w - 256 if w >= 1024 else w
                    x11 = x11p.tile([128, w], F32R, name=f"x11{k}_{lo}", tag="x11")
                    nc.gpsimd.tensor_copy(x11[:, :], xk[:, :])
                    xl = xlp.tile([128, w], F32R, name=f"xl{k}_{lo}", tag="xl")
                    nc.vector.tensor_sub(xl[:, :spl], xk[:, :spl], x11[:, :spl])
                    if spl < w:
                        nc.gpsimd.tensor_sub(
                            xl[:, spl:], xk[:, spl:], x11[:, spl:]
                        )
                    if PRECISE_B:
                        # m = s*x^2 in f32; hi-part = round11(m) on Pool;
                        # lo-part = m - hi (exact) on DVE. Both pass the PE
                        # untruncated.
                        mf = mfp.tile([128, w], F32, name=f"mf{k}_{lo}", tag="mf")
                        nc.scalar.activation(
                            mf[:, :], xk[:, :], AF.Square, scale=sqs_sb[:, k:k + 1]
                        )
                        mr = mrp.tile([128, w], F32R, name=f"mr{k}_{lo}", tag="mr")
                        nc.gpsimd.tensor_copy(mr[:, :], mf[:, :])
                        ml = mlp.tile([128, w], F32R, name=f"ml{k}_{lo}", tag="ml")
                        nc.vector.tensor_sub(ml[:, :spl], mf[:, :spl], mr[:, :spl])
                        if spl < w:
                            nc.gpsimd.tensor_sub(
                                ml[:, spl:], mf[:, spl:], mr[:, spl:]
                            )
                    else:
                        mr = mrp.tile([128, w], F32R, name=f"mr{k}_{lo}", tag="mr")
                        nc.scalar.activation(
                            mr[:, :], xk[:, :], AF.Square, scale=sqs_sb[:, k:k + 1]
                        )
                        ml = None

                    chunks = range(lo // NCHUNK, hi // NCHUNK)
                    # x11-dependent matmuls first (ready earliest), then xl/m
                    for n in chunks:
                        sl = slice(n * NCHUNK - lo, (n + 1) * NCHUNK - lo)
                        nc.tensor.matmul(
                            psumA[n][:, :], vw11_k, x11[:, sl],
                            start=first, stop=False,
                        )
                        nc.tensor.matmul(
                            psumA[n][:, :], vwl_k, x11[:, sl],
                            start=False, stop=False,
                        )
                    for n in chunks:
                        sl = slice(n * NCHUNK - lo, (n + 1) * NCHUNK - lo)
                        nc.tensor.matmul(
                            psumA[n][:, :], vw11_k, xl[:, sl],
                            start=False, stop=last,
                        )
                    for n in chunks:
                        sl = slice(n * NCHUNK - lo, (n + 1) * NCHUNK - lo)
                        nc.tensor.matmul(
                            psumB[n][:, :], ones_sb[:, :], mr[:, sl],
                            start=first, stop=(last and not PRECISE_B),
                        )
                    if PRECISE_B:
                        for n in chunks:
                            sl = slice(n * NCHUNK - lo, (n + 1) * NCHUNK - lo)
                            nc.tensor.matmul(
                                psumB[n][:, :], ones_sb[:, :], ml[:, sl],
                                start=False, stop=last,
                            )

            # First and last stripes in quarters: the first fills the pipeline
            # quickly; the last lets each chunk close its accumulation (and
            # start its epilogue) without waiting for the whole-stripe subs.
            quarters = [(i * NCHUNK, (i + 1) * NCHUNK) for i in range(NCHUNKS)]
            process(0, quarters)
            for k in range(1, KTILES - 1):
                process(k, [(0, BS)])
            process(KTILES - 1, quarters)

            # ---- epilogue: batch same-function ACT ops to avoid table reloads ----
            rhss, psumCs = [], []
            for n in range(NCHUNKS):
                # rows 0..63 = (xv)^2, 64 = lin, 65..95 zero, 96 = Bq
                rhs = redrhs.tile([97, NCHUNK], F32, name=f"rhs{n}", tag="rhs")
                nc.scalar.activation(rhs[0:EMBED, :], psumA[n][0:EMBED, :], AF.Square)
                nc.gpsimd.memset(rhs[64:96, :], 0.0)
                rhss.append(rhs)
            for n in range(NCHUNKS):
                nc.vector.tensor_copy(rhss[n][64:65, :], psumA[n][EMBED:M, :])
                nc.vector.tensor_copy(rhss[n][96:97, :], psumB[n][:, :])
            for n in range(NCHUNKS):
                # reuse a freed psumA slot (all psumA released after rhs built)
                psumC = psA.tile([1, NCHUNK], F32, name=f"psumC{n}", tag="psumA")
                nc.tensor.matmul(
                    psumC[:, :], red_sb[:, :], rhss[n][:, :], start=True, stop=True
                )
                out_sb = outp.tile([1, NCHUNK], F32, name=f"out{n}", tag="out")
                nc.scalar.activation(
                    out_sb[:, :], psumC[:, :], AF.Sigmoid, bias=b_sb[0:1, 0:1]
                )
                nc.gpsimd.dma_start(y[n:n + 1, :], out_sb[:, :])

    nc.compile()
    return nc


_NC_CACHE = None


def _prep_inputs(x, w, b, v):
    x = np.ascontiguousarray(x, dtype=np.float32)
    w = np.asarray(w, dtype=np.float32).reshape(FIELD, 1)
    v = np.asarray(v, dtype=np.float32)
    b0 = float(np.asarray(b, dtype=np.float32).reshape(-1)[0])

    s64 = (v.astype(np.float64) ** 2).sum(axis=1)
    sqs = np.sqrt(s64).astype(np.float32)
    vw = np.concatenate([v, w], axis=1).astype(np.float32)  # [FIELD, M]

    # hi/lo split on the f32r (11-mantissa-bit) grid; vw11 + vwl == vw to
    # within half an f32 ulp, both pieces pass through the PE unaltered.
    ui = vw.view(np.uint32).astype(np.uint64)
    r = (((ui + (1 << 11)) >> 12) << 12) & 0xFFFFFFFF
    vw11 = r.astype(np.uint32).view(np.float32)
    ui_l = ((vw.astype(np.float64) - vw11).astype(np.float32)
            .view(np.uint32).astype(np.uint64))
    r_l = (((ui_l + (1 << 11)) >> 12) << 12) & 0xFFFFFFFF
    vwl = r_l.astype(np.uint32).view(np.float32)

    def pack(a):  # [FIELD, M] -> [128, KTILES*M] SBUF image
        return np.ascontiguousarray(
            a.reshape(KTILES, 128, M).transpose(1, 0, 2).reshape(128, KTILES * M)
        )

    vw11i, vwli = pack(vw11), pack(vwl)
    sqsi = np.ascontiguousarray(sqs.reshape(KTILES, 128).T)

    red = np.zeros((97, 1), np.float32)
    red[0:EMBED, 0] = 0.5
    red[EMBED, 0] = 1.0
    red[96, 0] = -0.5
    ones = np.ones((128, 1), np.float32)
    bvec = np.full((1, 1), b0, np.float32)

    in_maps = []
    for c in range(NCORES):
        xt_c = np.ascontiguousarray(x[c * BS:(c + 1) * BS, :].T)
        in_maps.append({
            "xt": xt_c, "vw11i": vw11i, "vwli": vwli, "sqsi": sqsi,
            "red": red, "ones": ones, "bvec": bvec,
        })
    return in_maps


def _run(x, w, b, v, **spmd_kwargs):
    global _NC_CACHE
    if _NC_CACHE is None:
        _NC_CACHE = _build_nc()
    nc = _NC_CACHE

    in_maps = _prep_inputs(x, w, b, v)
    res = run_bass_kernel_spmd(nc, in_maps, list(range(NCORES)), **spmd_kwargs)
    out = np.concatenate(
        [res.results[c]["y"].reshape(BS) for c in range(NCORES)]
    )
    return out.reshape(BATCH, 1).astype(np.float32), res


def kernel(x, w, b, v):
    out, _ = _run(x, w, b, v)
    return out



# revision 2
# speedup vs baseline: 1.0748x; 1.0748x over previous
"""DeepFM forward kernel for 8 Trainium2 NeuronCores (Bass/Tile).

Math per batch row b (see reference.py):
    lin[b] = x[b] @ w + b0
    A[b]   = sum_k (x[b] @ v)_k^2
    Bq[b]  = sum_f s_f x[b,f]^2,  s_f = sum_k v[f,k]^2
    out[b] = sigmoid(lin[b] + 0.5*A[b] - 0.5*Bq[b])

Host-side transforms (parameter-only except the fp16 cast/transpose of x):
    sc = 4*sqrt(s);  xs = (x*sc) as fp16 (shipped, 2 B/elem);
    vw = [v | w]/sc as fp16 (stationary), so xs @ vw == x @ [v|w];
    m  = xs^2 = 16*s*x^2 on-chip, Bq = sum_f m / 16.

Per core (BS=2048 batch rows, fp16 single pass everywhere; tolerance is
2e-2 rel err, this lands ~6e-4):
  - DMA: x streams as stripe-pair transfers [128,2,2048] spread over all
    three DMA-capable queues (SP / scalar / pool) so transfers overlap;
    pair 0 leads with a quarter transfer and the vw head rides the pool
    queue first, so the PE starts by ~3us and never idles (a PE idle gap
    resets its clock ramp to half speed for 3us).
  - m-pipeline: custom fused DVE op SQSUM_ANT (mac = in0^2 + in1^2) does
    square+pair-add in one instruction; ACT squares pairs 5,6 (with GP
    adds) since DVE is the throughput limit.
  - PE, one continuous burst: 64 fp16 A-matmuls (xv+lin into psumA banks
    0-3; stripes 12-15 grouped per chunk so banks close staggered), 32
    fp16 B-matmuls with stationary -1/32 (psumB accumulates -0.5*Bq),
    and 4 reduce matmuls that accumulate 0.5*sum((xv)^2)+lin INTO psumB
    so the logit lands in psumB in place (no Bq copy).
  - Epilogue per chunk ((xv)^2 + lin copy): ACT activations for chunks
    0,1 straight from psum; DVE copy+square for 2,3 (HW: GPSIMD cannot
    access PSUM, vector ops may read only one PSUM operand). ACT
    sigmoids; y DMAs on the idle SP queue.
"""

import numpy as np

import concourse.bass as bass
import concourse.dve_ops as dve_ops
import concourse.tile as tile
from concourse import bacc, mybir
from concourse.bass_utils import run_bass_kernel_spmd
from concourse.dve_spec import Spec, Src0, Src1, lower, sq

BATCH, FIELD, EMBED = 16384, 2048, 64
NCORES = 8
BS = BATCH // NCORES       # 2048 batch rows per core
KT = FIELD // 128          # 16 contraction stripes
NP = KT // 2               # 8 stripe pairs
NG = NP // 2               # 4 DoubleRow mac groups (2 pairs each)
M = EMBED + 1              # 65 stationary cols: v plus w
CH = 512                   # psum bank free width (f32)
NCH = BS // CH             # 4 chunks

F32 = mybir.dt.float32
F32R = mybir.dt.float32r
F16 = mybir.dt.float16
F8 = mybir.dt.float8e4
AF = mybir.ActivationFunctionType
MULT = mybir.AluOpType.mult
ADD = mybir.AluOpType.add
DR = mybir.MatmulPerfMode.DoubleRowSwInterleave

ACT_PAIRS = (5, 6)      # pairs squared on ACT (+GP adds): DVE offload


def _register_sqsum():
    """Register the fused mac = in0^2 + in1^2 DVE op (idempotent)."""
    for op in dve_ops.OPS:
        if op.name == "SQSUM_ANT":
            return op
    spec = Spec(
        body=sq(Src0) + sq(Src1),
        reference=lambda in0, in1, s0, s1, imm2: in0 * in0 + in1 * in1,
    )
    shas = {}
    for ver in ("v3", "v4"):
        tmp = dve_ops.DveOpSpec(
            name="SQSUM_ANT", opcode=31, uops=lower(spec, ver=ver), rd1_en=True)
        shas[ver] = tmp.sha(ver)
    op = dve_ops.DveOp("SQSUM_ANT", spec, subdim=False, uops_sha=shas)
    dve_ops.OPS.append(op)
    dve_ops.CUSTOM_DVE_SPECS[op.name] = op.spec
    dve_ops._SUB_OPCODE_FOR_NAME[op.name] = (
        dve_ops._CUSTOM_DVE_ROW_BASE + len(dve_ops.OPS) - 1)
    return op


SQSUM = _register_sqsum()


def _build_nc():
    nc = bacc.Bacc("TRN2", target_bir_lowering=False, debug=False)

    # x stripe pairs: [pair, partition, stripe-in-pair, batch]
    xtp = nc.declare_dram_parameter("xtp", [NP, 128, 2, BS], F16, isOutput=False)
    vwh = nc.declare_dram_parameter("vwh", [128, 2 * M], F16, isOutput=False)
    vwr = nc.declare_dram_parameter("vwr", [128, (KT - 2) * M], F16, isOutput=False)
    negd = nc.declare_dram_parameter("negd", [128, 1], F16, isOutput=False)
    redd = nc.declare_dram_parameter("redd", [65, 1], F32R, isOutput=False)
    bd = nc.declare_dram_parameter("bd", [1, 1], F32, isOutput=False)
    y = nc.declare_dram_parameter("y", [NCH, CH], F32, isOutput=True)

    with tile.TileContext(nc) as tc:
        with (
            tc.tile_pool(name="consts", bufs=1) as consts,
            tc.tile_pool(name="xin", bufs=NP) as xin,
            tc.tile_pool(name="tt", bufs=1) as ttp,
            tc.tile_pool(name="mac", bufs=NG) as macp,
            tc.tile_pool(name="rhs", bufs=4) as rhsp,
            tc.tile_pool(name="outp", bufs=4) as outp,
            tc.tile_pool(name="psA", bufs=4, space="PSUM") as psA,
            tc.tile_pool(name="psB", bufs=4, space="PSUM") as psB,
        ):
            # ---- input DMA over all three queues ----
            vw_sb = consts.tile([128, KT * M], F16)
            nc.gpsimd.dma_start(vw_sb[:, 0:2 * M], vwh[:, :])  # PE gate: first
            neg_sb = consts.tile([128, 1], F16)
            nc.gpsimd.dma_start(neg_sb[:, :], negd[:, :])
            b_sb = consts.tile([1, 1], F32)
            nc.gpsimd.dma_start(b_sb[:, :], bd[:, :])
            red_sb = consts.tile([65, 1], F32R)
            nc.gpsimd.dma_start(red_sb[:, :], redd[:, :])
            nc.gpsimd.dma_start(vw_sb[:, 2 * M:], vwr[:, :])

            xkp = [xin.tile([128, 2, BS], F16, name=f"xp{j}", tag="xp")
                   for j in range(NP)]
            q4 = BS // 4
            nc.sync.dma_start(xkp[0][:, :, 0:q4], xtp[0, :, :, 0:q4])
            nc.scalar.dma_start(xkp[0][:, :, q4:], xtp[0, :, :, q4:])
            for j, q in ((1, nc.sync), (2, nc.scalar), (3, nc.gpsimd),
                         (4, nc.sync), (5, nc.scalar), (6, nc.gpsimd),
                         (7, nc.sync)):
                q.dma_start(xkp[j][:, :, :], xtp[j, :, :, :])

            psumA = [psA.tile([M, CH], F32, name=f"psA{n}", tag="psA")
                     for n in range(NCH)]
            psumB = [psB.tile([1, CH], F32, name=f"psB{n}", tag="psB")
                     for n in range(NCH)]
            # mac groups: [:, 0, :] = pair 2g, [:, 1, :] = pair 2g+1
            macg = [macp.tile([128, 2, BS], F16, name=f"mg{g}", tag="mg")
                    for g in range(NG)]

            def emit_mac(j):
                g, h = j // 2, j % 2
                if j in ACT_PAIRS:
                    t = ttp.tile([128, 2, BS], F16, name=f"t{j}", tag="t")
                    nc.scalar.activation(t[:, :, :], xkp[j][:, :, :], AF.Square)
                    nc.gpsimd.tensor_tensor(
                        macg[g][:, h, :], t[:, 0, :], t[:, 1, :], ADD)
                else:
                    nc.vector._custom_dve(
                        SQSUM, out=macg[g][:, h, :],
                        in0=xkp[j][:, 0, :], in1=xkp[j][:, 1, :])

            for j in (0, 1, 2, 3, 4, 5, 6, 7):
                emit_mac(j)

            def emit_A(k, chunks=None):
                vw_k = vw_sb[:, k * M:(k + 1) * M]
                j, h = k // 2, k % 2
                for n in (range(NCH) if chunks is None else chunks):
                    nc.tensor.matmul(
                        psumA[n][:, :], vw_k, xkp[j][:, h, n * CH:(n + 1) * CH],
                        start=(k == 0), stop=(k == KT - 1))

            def emit_Bg(g, chunks=None):
                # stationary -1/32 (fp16): psumB accumulates -0.5*Bq
                for n in (range(NCH) if chunks is None else chunks):
                    for h in (0, 1):
                        nc.tensor.matmul(
                            psumB[n][:, :], neg_sb[:, :],
                            macg[g][:, h, n * CH:(n + 1) * CH],
                            start=(g == 0 and h == 0), stop=False)

            # rhs rows: 0..63 = (xv)^2, 64 = lin
            rhs = [rhsp.tile([65, CH], F32R, name=f"rhs{n}", tag="rhs")
                   for n in range(NCH)]
            # hardware allows ONE psum operand per vector op: squares of
            # psumA go through SQSUM with a zero [P,1] broadcast second
            # operand (DVE), or ACT activation Square (chunk 0).
            zero64 = consts.tile([64, 1], F32)
            nc.gpsimd.memset(zero64[:, :], 0.0)

            def emit_epi_sq(n):
                # HW limits: GPSIMD can't touch PSUM; vector ops may read
                # only one PSUM operand. ACT squares chunks 0,1 straight
                # from psum; DVE copies 2,3 to sbuf and squares there.
                if n in (0, 1):
                    nc.scalar.activation(rhs[n][0:64, :], psumA[n][0:64, :],
                                         AF.Square)
                    nc.scalar.activation(rhs[n][64:65, :], psumA[n][64:65, :],
                                         AF.Copy)
                    return
                tmp = rhsp.tile([65, CH], F32, name=f"tmp{n}", tag=f"tmp{n}")
                nc.vector.tensor_copy(tmp[:, :], psumA[n][0:65, :])
                nc.vector.tensor_tensor(rhs[n][0:64, :], tmp[0:64, :],
                                        tmp[0:64, :], MULT)
                nc.vector.tensor_copy(rhs[n][64:65, :], tmp[64:65, :])

            def emit_epi_out(n):
                nc.tensor.matmul(psumB[n][:, :], red_sb[:, :], rhs[n][:, :],
                                 start=False, stop=True)
                out_sb = outp.tile([1, CH], F32, name=f"out{n}", tag="out")
                nc.scalar.activation(out_sb[:, :], psumB[n][:, :], AF.Sigmoid,
                                     bias=b_sb[0:1, 0:1])
                nc.sync.dma_start(y[n:n + 1, :], out_sb[:, :])

            # ---- PE stream: stripes 0-11 chunk-parallel, then per-chunk
            # [A12..A15] groups so psumA banks close staggered and early;
            # the last B group interleaves with the reduce matmuls.
            for k in range(12):
                emit_A(k)
                if k == 11:
                    emit_Bg(0)
            for c in range(NCH):
                for k in (12, 13, 14, 15):
                    emit_A(k, chunks=[c])
                emit_epi_sq(c)
                if c == 0:
                    emit_Bg(1)
                elif c == 1:
                    emit_Bg(2)
            for c in range(NCH):
                emit_Bg(3, chunks=[c])
                emit_epi_out(c)

    nc.compile()
    return nc


_NC_CACHE = None


def _prep_inputs(x, w, b, v):
    x = np.asarray(x, dtype=np.float32)
    w = np.asarray(w, dtype=np.float32).reshape(FIELD, 1)
    v = np.asarray(v, dtype=np.float32)
    b0 = float(np.asarray(b, dtype=np.float32).reshape(-1)[0])

    s = (v.astype(np.float64) ** 2).sum(axis=1)
    sc = 4.0 * np.sqrt(s)                                   # [FIELD]
    vw = np.concatenate([v, w], axis=1).astype(np.float64)  # [FIELD, M]
    vw16 = (vw / sc[:, None]).astype(np.float16)

    # SBUF image [128, KT*M]: partition p holds stripe-major blocks of M cols
    vwi = np.ascontiguousarray(
        vw16.reshape(KT, 128, M).transpose(1, 0, 2).reshape(128, KT * M))
    vwh = np.ascontiguousarray(vwi[:, :2 * M])
    vwr = np.ascontiguousarray(vwi[:, 2 * M:])

    red = np.zeros((65, 1), np.float32)
    red[0:EMBED, 0] = 0.5
    red[EMBED, 0] = 1.0
    negd = np.full((128, 1), -1.0 / 32.0, np.float16)
    bd = np.full((1, 1), b0, np.float32)

    xs = (x.astype(np.float64) * sc[None, :]).astype(np.float16)
    in_maps = []
    for c in range(NCORES):
        xt_c = xs[c * BS:(c + 1) * BS, :].T          # [FIELD, BS]
        xtp_c = np.ascontiguousarray(
            xt_c.reshape(NP, 2, 128, BS).transpose(0, 2, 1, 3))
        in_maps.append({
            "xtp": xtp_c, "vwh": vwh, "vwr": vwr, "negd": negd,
            "redd": red, "bd": bd,
        })
    return in_maps


def _run(x, w, b, v, **spmd_kwargs):
    global _NC_CACHE
    if _NC_CACHE is None:
        _NC_CACHE = _build_nc()
    nc = _NC_CACHE

    in_maps = _prep_inputs(x, w, b, v)
    res = run_bass_kernel_spmd(nc, in_maps, list(range(NCORES)), **spmd_kwargs)
    out = np.concatenate(
        [res.results[c]["y"].reshape(BS) for c in range(NCORES)]
    )
    return out.reshape(BATCH, 1).astype(np.float32), res


def kernel(x, w, b, v):
    out, _ = _run(x, w, b, v)
    return out


# revision 3
# speedup vs baseline: 1.1680x; 1.0867x over previous
"""DeepFM forward kernel for 8 Trainium2 NeuronCores (Bass/Tile).

Math per batch row b (see reference.py):
    lin[b] = x[b] @ w + b0
    A[b]   = sum_k (x[b] @ v)_k^2
    Bq[b]  = sum_f s_f x[b,f]^2,  s_f = sum_k v[f,k]^2
    out[b] = sigmoid(lin[b] + 0.5*A[b] - 0.5*Bq[b])

Host-side transforms (parameter-only except the fp16 cast/transpose of x):
    sc = 4*sqrt(s);  xs = (x*sc) as fp16 (shipped, 2 B/elem);
    vw = [v | w]/sc as fp16 (stationary), so xs @ vw == x @ [v|w];
    m  = xs^2 = 16*s*x^2 on-chip, Bq = sum_f m / 16.

Per core (BS=2048 batch rows, fp16 single pass everywhere; tolerance is
2e-2 rel err, this lands ~6e-4):
  - DMA: x streams as stripe-pair transfers [128,2,2048] spread over all
    three DMA-capable queues (SP / scalar / pool) so transfers overlap;
    pair 0 leads with a quarter transfer and the vw head rides the pool
    queue first, so the PE starts by ~3us and never idles (a PE idle gap
    resets its clock ramp to half speed for 3us).
  - m-pipeline: custom fused DVE op SQSUM_ANT (mac = in0^2 + in1^2) does
    square+pair-add in one instruction; ACT squares pairs 5,6 (with GP
    adds) since DVE is the throughput limit.
  - PE, one continuous burst: 64 fp16 A-matmuls (xv+lin into psumA banks
    0-3; stripes 12-15 grouped per chunk so banks close staggered), 32
    fp16 B-matmuls with stationary -1/32 (psumB accumulates -0.5*Bq),
    and 4 reduce matmuls that accumulate 0.5*sum((xv)^2)+lin INTO psumB
    so the logit lands in psumB in place (no Bq copy).
  - Epilogue per chunk ((xv)^2 + lin copy): ACT activations for chunks
    0,1 straight from psum; DVE copy+square for 2,3 (HW: GPSIMD cannot
    access PSUM, vector ops may read only one PSUM operand). ACT
    sigmoids; y DMAs on the idle SP queue.
"""

import numpy as np

import concourse.bass as bass
import concourse.dve_ops as dve_ops
import concourse.tile as tile
from concourse import bacc, mybir
from concourse.bass_utils import run_bass_kernel_spmd
from concourse.dve_spec import Spec, Src0, Src1, lower, sq

BATCH, FIELD, EMBED = 16384, 2048, 64
NCORES = 8
BS = BATCH // NCORES       # 2048 batch rows per core
KT = FIELD // 128          # 16 contraction stripes
NP = KT // 2               # 8 stripe pairs
NG = NP // 2               # 4 DoubleRow mac groups (2 pairs each)
M = EMBED + 1              # 65 stationary cols: v plus w
CH = 512                   # psum bank free width (f32)
NCH = BS // CH             # 4 chunks

F32 = mybir.dt.float32
F32R = mybir.dt.float32r
F16 = mybir.dt.float16
F8 = mybir.dt.float8e4
AF = mybir.ActivationFunctionType
MULT = mybir.AluOpType.mult
ADD = mybir.AluOpType.add
DR = mybir.MatmulPerfMode.DoubleRowSwInterleave

ACT_PAIRS = (5, 6)      # pairs squared on ACT (+GP adds): DVE offload


def _register_sqsum():
    """Register the fused mac = in0^2 + in1^2 DVE op (idempotent)."""
    for op in dve_ops.OPS:
        if op.name == "SQSUM_ANT":
            return op
    spec = Spec(
        body=sq(Src0) + sq(Src1),
        reference=lambda in0, in1, s0, s1, imm2: in0 * in0 + in1 * in1,
    )
    shas = {}
    for ver in ("v3", "v4"):
        tmp = dve_ops.DveOpSpec(
            name="SQSUM_ANT", opcode=31, uops=lower(spec, ver=ver), rd1_en=True)
        shas[ver] = tmp.sha(ver)
    op = dve_ops.DveOp("SQSUM_ANT", spec, subdim=False, uops_sha=shas)
    dve_ops.OPS.append(op)
    dve_ops.CUSTOM_DVE_SPECS[op.name] = op.spec
    dve_ops._SUB_OPCODE_FOR_NAME[op.name] = (
        dve_ops._CUSTOM_DVE_ROW_BASE + len(dve_ops.OPS) - 1)
    return op


SQSUM = _register_sqsum()


def _build_nc():
    nc = bacc.Bacc("TRN2", target_bir_lowering=False, debug=False)

    # x stripe pairs: [pair, partition, stripe-in-pair, batch]
    xtp = nc.declare_dram_parameter("xtp", [NP, 128, 2, BS], F16, isOutput=False)
    vwh = nc.declare_dram_parameter("vwh", [128, 2 * M], F16, isOutput=False)
    vwr = nc.declare_dram_parameter("vwr", [128, (KT - 2) * M], F16, isOutput=False)
    negd = nc.declare_dram_parameter("negd", [128, 1], F16, isOutput=False)
    redd = nc.declare_dram_parameter("redd", [65, 1], F32R, isOutput=False)
    bd = nc.declare_dram_parameter("bd", [1, 1], F32, isOutput=False)
    y = nc.declare_dram_parameter("y", [NCH, CH], F32, isOutput=True)

    with tile.TileContext(nc) as tc:
        with (
            tc.tile_pool(name="consts", bufs=1) as consts,
            tc.tile_pool(name="xin", bufs=NP) as xin,
            tc.tile_pool(name="tt", bufs=1) as ttp,
            tc.tile_pool(name="mac", bufs=NG) as macp,
            tc.tile_pool(name="rhs", bufs=4) as rhsp,
            tc.tile_pool(name="outp", bufs=4) as outp,
            tc.tile_pool(name="psA", bufs=4, space="PSUM") as psA,
            tc.tile_pool(name="psB", bufs=4, space="PSUM") as psB,
        ):
            # ---- input DMA over all three queues ----
            vw_sb = consts.tile([128, KT * M], F16)
            nc.gpsimd.dma_start(vw_sb[:, 0:2 * M], vwh[:, :])  # PE gate: first
            neg_sb = consts.tile([128, 1], F16)
            nc.gpsimd.dma_start(neg_sb[:, :], negd[:, :])
            b_sb = consts.tile([1, 1], F32)
            nc.gpsimd.dma_start(b_sb[:, :], bd[:, :])
            red_sb = consts.tile([65, 1], F32R)
            nc.gpsimd.dma_start(red_sb[:, :], redd[:, :])
            nc.gpsimd.dma_start(vw_sb[:, 2 * M:], vwr[:, :])

            xkp = [xin.tile([128, 2, BS], F16, name=f"xp{j}", tag="xp")
                   for j in range(NP)]
            h2 = BS // 2
            nc.sync.dma_start(xkp[0][:, :, 0:h2], xtp[0, :, :, 0:h2])
            nc.scalar.dma_start(xkp[0][:, :, h2:], xtp[0, :, :, h2:])
            for j, q in ((1, nc.sync), (2, nc.scalar), (3, nc.gpsimd),
                         (4, nc.sync), (5, nc.scalar), (6, nc.gpsimd),
                         (7, nc.sync)):
                q.dma_start(xkp[j][:, :, :], xtp[j, :, :, :])

            psumA = [psA.tile([M, CH], F32, name=f"psA{n}", tag="psA")
                     for n in range(NCH)]
            psumB = [psB.tile([1, CH], F32, name=f"psB{n}", tag="psB")
                     for n in range(NCH)]
            # mac groups: [:, 0, :] = pair 2g, [:, 1, :] = pair 2g+1
            macg = [macp.tile([128, 2, BS], F16, name=f"mg{g}", tag="mg")
                    for g in range(NG)]
            # groups 0,1 (early DVE macs) get quad-summed by the idle GP:
            # one B-matmul instead of two for those groups
            mq = [macp.tile([128, BS], F16, name=f"mq{g}", tag="mq")
                  for g in range(2)]

            def emit_mac(j):
                g, h = j // 2, j % 2
                if j in ACT_PAIRS:
                    t = ttp.tile([128, 2, BS], F16, name=f"t{j}", tag="t")
                    nc.scalar.activation(t[:, :, :], xkp[j][:, :, :], AF.Square)
                    # pair 6's add on DVE (free by then, and faster than GP)
                    e = nc.vector if j == 6 else nc.gpsimd
                    e.tensor_tensor(
                        macg[g][:, h, :], t[:, 0, :], t[:, 1, :], ADD)
                else:
                    nc.vector._custom_dve(
                        SQSUM, out=macg[g][:, h, :],
                        in0=xkp[j][:, 0, :], in1=xkp[j][:, 1, :])

            for j in (0, 1, 2, 3, 4, 5, 6, 7):
                emit_mac(j)
                if j == 1:
                    nc.gpsimd.tensor_tensor(mq[0][:, :], macg[0][:, 0, :],
                                            macg[0][:, 1, :], ADD)
                elif j == 3:
                    nc.gpsimd.tensor_tensor(mq[1][:, :], macg[1][:, 0, :],
                                            macg[1][:, 1, :], ADD)

            def emit_A(k, chunks=None):
                vw_k = vw_sb[:, k * M:(k + 1) * M]
                j, h = k // 2, k % 2
                for n in (range(NCH) if chunks is None else chunks):
                    nc.tensor.matmul(
                        psumA[n][:, :], vw_k, xkp[j][:, h, n * CH:(n + 1) * CH],
                        start=(k == 0), stop=(k == KT - 1))

            def emit_Bg(g, chunks=None):
                # stationary -1/32 (fp16): psumB accumulates -0.5*Bq.
                # Groups 0,1 ride their GP quad-sum (one matmul); 2,3 use
                # the pair halves directly (their macs land late).
                for n in (range(NCH) if chunks is None else chunks):
                    if g < 2:
                        nc.tensor.matmul(
                            psumB[n][:, :], neg_sb[:, :],
                            mq[g][:, n * CH:(n + 1) * CH],
                            start=(g == 0), stop=False)
                    else:
                        for h in (0, 1):
                            nc.tensor.matmul(
                                psumB[n][:, :], neg_sb[:, :],
                                macg[g][:, h, n * CH:(n + 1) * CH],
                                start=False, stop=False)

            # rhs rows: 0..63 = (xv)^2, 64 = lin
            rhs = [rhsp.tile([65, CH], F32R, name=f"rhs{n}", tag="rhs")
                   for n in range(NCH)]
            # hardware allows ONE psum operand per vector op: squares of
            # psumA go through SQSUM with a zero [P,1] broadcast second
            # operand (DVE), or ACT activation Square (chunk 0).
            zero64 = consts.tile([64, 1], F32)
            nc.gpsimd.memset(zero64[:, :], 0.0)

            def emit_epi_sq(n):
                # HW limits: GPSIMD can't touch PSUM; vector ops may read
                # only one PSUM operand. ACT squares chunks 0,1 straight
                # from psum; DVE copies 2,3 to sbuf and squares there.
                if n in (0, 1):
                    nc.scalar.activation(rhs[n][0:64, :], psumA[n][0:64, :],
                                         AF.Square)
                    nc.scalar.activation(rhs[n][64:65, :], psumA[n][64:65, :],
                                         AF.Copy)
                    return
                tmp = rhsp.tile([65, CH], F32, name=f"tmp{n}", tag=f"tmp{n}")
                nc.vector.tensor_copy(tmp[:, :], psumA[n][0:65, :])
                nc.vector.tensor_tensor(rhs[n][0:64, :], tmp[0:64, :],
                                        tmp[0:64, :], MULT)
                nc.vector.tensor_copy(rhs[n][64:65, :], tmp[64:65, :])

            def emit_epi_out(n):
                nc.tensor.matmul(psumB[n][:, :], red_sb[:, :], rhs[n][:, :],
                                 start=False, stop=True)
                out_sb = outp.tile([1, CH], F32, name=f"out{n}", tag="out")
                nc.scalar.activation(out_sb[:, :], psumB[n][:, :], AF.Sigmoid,
                                     bias=b_sb[0:1, 0:1])
                nc.sync.dma_start(y[n:n + 1, :], out_sb[:, :])

            # ---- PE stream: stripes 0-11 chunk-parallel, then per-chunk
            # [A12..A15] groups so psumA banks close staggered and early;
            # the last B group interleaves with the reduce matmuls.
            for k in range(12):
                emit_A(k)
                if k == 11:
                    emit_Bg(0)
            for c in range(NCH):
                for k in (12, 13, 14, 15):
                    emit_A(k, chunks=[c])
                emit_epi_sq(c)
                if c == 0:
                    emit_Bg(1)
                elif c == 1:
                    emit_Bg(2)
            for c in range(NCH):
                emit_Bg(3, chunks=[c])
                emit_epi_out(c)

    nc.compile()
    return nc


_NC_CACHE = None


def _prep_inputs(x, w, b, v):
    x = np.asarray(x, dtype=np.float32)
    w = np.asarray(w, dtype=np.float32).reshape(FIELD, 1)
    v = np.asarray(v, dtype=np.float32)
    b0 = float(np.asarray(b, dtype=np.float32).reshape(-1)[0])

    s = (v.astype(np.float64) ** 2).sum(axis=1)
    sc = 4.0 * np.sqrt(s)                                   # [FIELD]
    vw = np.concatenate([v, w], axis=1).astype(np.float64)  # [FIELD, M]
    vw16 = (vw / sc[:, None]).astype(np.float16)

    # SBUF image [128, KT*M]: partition p holds stripe-major blocks of M cols
    vwi = np.ascontiguousarray(
        vw16.reshape(KT, 128, M).transpose(1, 0, 2).reshape(128, KT * M))
    vwh = np.ascontiguousarray(vwi[:, :2 * M])
    vwr = np.ascontiguousarray(vwi[:, 2 * M:])

    red = np.zeros((65, 1), np.float32)
    red[0:EMBED, 0] = 0.5
    red[EMBED, 0] = 1.0
    negd = np.full((128, 1), -1.0 / 32.0, np.float16)
    bd = np.full((1, 1), b0, np.float32)

    xs = (x.astype(np.float64) * sc[None, :]).astype(np.float16)
    in_maps = []
    for c in range(NCORES):
        xt_c = xs[c * BS:(c + 1) * BS, :].T          # [FIELD, BS]
        xtp_c = np.ascontiguousarray(
            xt_c.reshape(NP, 2, 128, BS).transpose(0, 2, 1, 3))
        in_maps.append({
            "xtp": xtp_c, "vwh": vwh, "vwr": vwr, "negd": negd,
            "redd": red, "bd": bd,
        })
    return in_maps


def _run(x, w, b, v, **spmd_kwargs):
    global _NC_CACHE
    if _NC_CACHE is None:
        _NC_CACHE = _build_nc()
    nc = _NC_CACHE

    in_maps = _prep_inputs(x, w, b, v)
    res = run_bass_kernel_spmd(nc, in_maps, list(range(NCORES)), **spmd_kwargs)
    out = np.concatenate(
        [res.results[c]["y"].reshape(BS) for c in range(NCORES)]
    )
    return out.reshape(BATCH, 1).astype(np.float32), res


def kernel(x, w, b, v):
    out, _ = _run(x, w, b, v)
    return out
